# revision 1
# baseline (speedup 1.0000x reference)
"""MetaDGCRU Trainium2 kernel.

Problem (hardcoded shapes): B=8, N=400, INPUT_DIM=2, HIDDEN=64,
GRAPH_NUM=2, HOP_K=2, NODE_EMB_DIM=16, IN_FEAT=66, I_DIM=330.

Sharding: data-parallel over batch B across the 8 NeuronCores (one batch
element per core); weight pools replicated, per-graph adjacencies sharded
with their batch.

Per-core computation (feature-on-partition / "transposed" layouts):
  xsT = [x;state].T                                    [66, 400]
  hops transposed-out:  YT = lhsT(X_nat).T @ AT        (PE, 4 m-chunks)
  hT = concat pieces -> 3 tiles of [128, 400] (i padded 330->384)
  gT[(d,i), n] = embT[d,n] * hT[i,n]                   (DVE+GPS, 48 chunks)
  zrT = bias(start=True, K=16) + sum_c Wg[c].T @ gT[c] (PE, 48 + 1 MMs)
  z,r = sigmoid(zrT);  xrsT = [xT; rT*stateT];  repeat -> hcT = tanh(...)
  out hT = hcT + zT*(stateT - hcT)                     [64, 400] f32

DMA strategy: HWDGE rings are FIFO per engine, so ordering is by emission:
the SP ring streams adjacency first, then embrep/Wg quarters interleaved in
the order compute consumes them; the ACT ring carries the small constants
and the mid-kernel piece/shift DMAs so they never queue behind bulk weights.
"""

import os

os.environ.setdefault("MYCRO_LOCAL_CACHE", "1")

import numpy as np
import ml_dtypes

B, N = 8, 400
INPUT_DIM, HIDDEN = 2, 64
GRAPH_NUM, HOP_K = 2, 2
D_EMB = 16
IN_FEAT = INPUT_DIM + HIDDEN               # 66
I_DIM = (GRAPH_NUM * HOP_K + 1) * IN_FEAT  # 330
KCH = 3                                    # i-chunks per d (128 each)
I_PAD = KCH * 128                          # 384
NCH = D_EMB * KCH                          # 48 total K chunks
O_G = 2 * HIDDEN                           # 128 gate out (z|r)
O_C = HIDDEN                               # 64 candidate out
NPAD = 512                                 # node dim padded for clean DMA packing

BF16 = ml_dtypes.bfloat16
MCHUNKS = [(0, 128), (128, 128), (256, 128), (384, 16)]  # node-dim chunking
QD = 4                                     # d's per streaming quarter

GPS_EVERY = 3  # every 3rd gT-build op runs on GpSimd instead of DVE

_CACHE = {}


def _emit(nc, tc, tile, mybir, ctx):
    """Emit the per-core kernel into TileContext tc."""
    dt = mybir.dt
    Sig = mybir.ActivationFunctionType.Sigmoid
    Tanh = mybir.ActivationFunctionType.Tanh
    Copy = mybir.ActivationFunctionType.Copy

    d_at = nc.dram_tensor("at", [GRAPH_NUM, 128, 3 * N], dt.bfloat16, kind="ExternalInput")
    d_at3 = nc.dram_tensor("at3", [GRAPH_NUM, 16, N], dt.bfloat16, kind="ExternalInput")
    d_xsT = nc.dram_tensor("xsT", [IN_FEAT, N], dt.bfloat16, kind="ExternalInput")
    d_xsnat = nc.dram_tensor("xsnat", [128, 4 * IN_FEAT], dt.bfloat16, kind="ExternalInput")
    d_state2 = nc.dram_tensor("state2", [2 * HIDDEN, N], dt.float32, kind="ExternalInput")
    d_embT = nc.dram_tensor("embT", [D_EMB, N], dt.bfloat16, kind="ExternalInput")
    d_embrep = nc.dram_tensor("embrep", [128, D_EMB * N], dt.bfloat16, kind="ExternalInput")
    d_wg = nc.dram_tensor("wg", [128, NCH * O_G], dt.bfloat16, kind="ExternalInput")
    d_wc = nc.dram_tensor("wc", [128, NCH * O_C], dt.bfloat16, kind="ExternalInput")
    d_bg = nc.dram_tensor("bg", [D_EMB, O_G], dt.bfloat16, kind="ExternalInput")
    d_bc = nc.dram_tensor("bc", [D_EMB, O_C], dt.bfloat16, kind="ExternalInput")
    d_ident = nc.dram_tensor("ident", [128, 128], dt.bfloat16, kind="ExternalInput")
    d_out = nc.dram_tensor("out", [HIDDEN, N], dt.float32, kind="ExternalOutput")

    cpool = ctx.enter_context(tc.tile_pool(name="const", bufs=1))
    hpool = ctx.enter_context(tc.tile_pool(name="hbuf", bufs=1))
    gpool = ctx.enter_context(tc.tile_pool(name="gbuf", bufs=1))
    spool = ctx.enter_context(tc.tile_pool(name="small", bufs=4))
    ppool = ctx.enter_context(tc.tile_pool(name="psum", bufs=2, space="PSUM"))
    ptp = ctx.enter_context(tc.tile_pool(name="psumT", bufs=2, space="PSUM"))
    pzr = ctx.enter_context(tc.tile_pool(name="psumZR", bufs=1, space="PSUM"))

    # ---- SP-ring priority inputs (FIFO: first emitted = first transferred) ----
    at_sb = []
    at3_sb = []
    for g in range(GRAPH_NUM):
        t = cpool.tile([128, 3 * N], dt.bfloat16, name=f"at{g}")
        nc.sync.dma_start(t[:], d_at[g, :, :])
        at_sb.append(t)
        t3 = cpool.tile([16, N], dt.bfloat16, name=f"at3_{g}")
        nc.sync.dma_start(t3[:], d_at3[g, :, :])
        at3_sb.append(t3)
    xsnat_sb = cpool.tile([128, 4 * IN_FEAT], dt.bfloat16, name="xsnat")
    nc.sync.dma_start(xsnat_sb[:], d_xsnat[:, :])

    # hT tiles + first pieces
    hT_g = [hpool.tile([128, N], dt.bfloat16, name=f"hTg{t}") for t in range(KCH)]
    hT_c = [hpool.tile([128, N], dt.bfloat16, name=f"hTc{t}") for t in range(KCH)]
    nc.vector.memset(hT_g[2][:, :], 0.0)
    nc.vector.memset(hT_c[2][:, :], 0.0)
    nc.sync.dma_start(hT_g[0][0:IN_FEAT, :], d_xsT[:, :])
    nc.sync.dma_start(hT_c[0][0:INPUT_DIM, :], d_xsT[0:INPUT_DIM, :])

    # ---- SP-ring bulk stream head (eq0/eq1); the tail yields to the gate
    # pieces (emitted later with a dep on the piece-1 DMA) ----
    embrep_sb = cpool.tile([128, D_EMB * N], dt.bfloat16, name="embrep")
    wg_sb = cpool.tile([128, NCH * O_G], dt.bfloat16, name="wg")
    wc_sb = cpool.tile([128, NCH * O_C], dt.bfloat16, name="wc")
    state2_sb = cpool.tile([2 * HIDDEN, N], dt.float32, name="state2")
    for q in range(3):
        e0 = q * QD * N
        nc.sync.dma_start(embrep_sb[:, e0:e0 + QD * N], d_embrep[:, e0:e0 + QD * N])

    def eq_dma(q):
        e0 = q * QD * N
        return nc.sync.dma_start(embrep_sb[:, e0:e0 + QD * N],
                                 d_embrep[:, e0:e0 + QD * N])

    def wg_dma(q):
        w0 = q * QD * KCH * O_G
        return nc.sync.dma_start(wg_sb[:, w0:w0 + QD * KCH * O_G],
                                 d_wg[:, w0:w0 + QD * KCH * O_G])

    def wc_dma(h):
        w0 = h * (NCH // 2) * O_C
        return nc.sync.dma_start(wc_sb[:, w0:w0 + (NCH // 2) * O_C],
                                 d_wc[:, w0:w0 + (NCH // 2) * O_C])

    bulk_groups = [
        [lambda: eq_dma(3), lambda: wg_dma(0)],
        [lambda: wg_dma(1)],
        [lambda: wg_dma(2)],
        [lambda: wg_dma(3), lambda: wc_dma(0), lambda: wc_dma(1),
         lambda: nc.sync.dma_start(state2_sb[:], d_state2[:, :])],
    ]

    def after_piece(piece_dma):
        from concourse.tile_rust import add_dep_helper
        if not bulk_groups:
            return
        group = bulk_groups.pop(0)
        first = group[0]()
        add_dep_helper(piece_dma.ins, first.ins, False,
                       "bulk group ordered after gate piece DMA")
        for fn in group[1:]:
            fn()

    # ---- ACT-ring small constants (separate HW queue from the bulk stream) ----
    embT_sb = cpool.tile([D_EMB, N], dt.bfloat16, name="embT")
    nc.scalar.dma_start(embT_sb[:], d_embT[:, :])
    ident_sb = cpool.tile([128, 128], dt.bfloat16, name="ident")
    nc.scalar.dma_start(ident_sb[:], d_ident[:, :])
    bg_sb = cpool.tile([D_EMB, O_G], dt.bfloat16, name="bg")
    nc.scalar.dma_start(bg_sb[:], d_bg[:, :])
    bc_sb = cpool.tile([D_EMB, O_C], dt.bfloat16, name="bc")
    nc.scalar.dma_start(bc_sb[:], d_bc[:, :])


    # dummy matmuls warm the PE (HAM) during the adjacency DMA wait
    ones_sb = cpool.tile([128, 512], dt.bfloat16, name="ones_sb")
    nc.vector.memset(ones_sb[:, :], 1.0)
    pbc = ctx.enter_context(tc.tile_pool(name="psumBC", bufs=2, space="PSUM"))
    for w in range(8):
        warm_ps = pbc.tile([128, 192], dt.float32, name=f"warm_ps{w}", tag="warmps", bufs=1)
        nc.tensor.matmul(warm_ps[:], ones_sb[:, 0:128], ones_sb[:, 0:192],
                         start=True, stop=True)

    # warm the ACT Copy table early (first pieceT copy needs it)
    warm = hpool.tile([1, 8], dt.float32, name="warm")
    nc.vector.memset(warm[:, :], 0.0)
    nc.scalar.activation(warm[:, 0:4], warm[:, 4:8], Copy)

    # gT buffer: 48 chunks of [128, N] side by side (shared gate/cand)
    gT = gpool.tile([128, NCH * N], dt.bfloat16, name="gT")

    def piece_to_hT(hT, piece, piece_ps, p_idx, cand=False):
        """Place piece [IN_FEAT, N] into hT tiles. Split pieces (1 and 3)
        put their leading spill rows in the next tile via a base-0 ACT copy
        straight from PSUM (the host W-pack permutation compensates); the
        main part goes via a single ACT-queue DMA. The candidate layout
        moves piece 1's main part to tile0[2:64] (rows 64:128 hold rs)."""
        if p_idx == 1:
            # spill rows 0:4 -> tile1[0:4]; main rows 4:66 -> tile0
            nc.scalar.activation(hT[1][0:4, :], piece_ps[0:4, :], Copy)
            dst = hT[0][2:64, :] if cand else hT[0][66:128, :]
            return [nc.sync.dma_start(dst, piece[4:IN_FEAT, :])]
        if p_idx == 3:
            # spill rows 0:8 -> tile2[0:8]; main rows 8:66 -> tile1[70:128]
            nc.scalar.activation(hT[2][0:8, :], piece_ps[0:8, :], Copy)
            return [nc.sync.dma_start(hT[1][70:128, :], piece[8:IN_FEAT, :])]
        r0 = IN_FEAT * p_idx
        t0, o0 = divmod(r0, 128)
        return [nc.sync.dma_start(hT[t0][o0:o0 + IN_FEAT, :], piece[:, :])]

    def hop(lhsT_of, g, name):
        """One propagation Y = A_g @ X, transposed out. lhsT_of(k)->AP [mlen,66]."""
        yt_ps = ppool.tile([IN_FEAT, N], dt.float32, name=f"ps_{name}", tag="hopps")
        for k, (moff, mlen) in enumerate(MCHUNKS):
            rhs = (at_sb[g][:, k * N:(k + 1) * N] if k < 3 else at3_sb[g][:, :])
            nc.tensor.matmul(
                yt_ps[:], lhsT_of(k), rhs,
                start=(k == 0), stop=(k == len(MCHUNKS) - 1),
            )
        yt = spool.tile([IN_FEAT, N], dt.bfloat16, name=f"yt_{name}", tag="hopsb")
        nc.scalar.activation(yt[:], yt_ps[:], Copy)
        return yt, yt_ps

    def nat_slicer(tl):
        return lambda k: tl[0:MCHUNKS[k][1], k * IN_FEAT:(k + 1) * IN_FEAT]

    def naturalize(yt, name):
        """PE-transpose YT [66, N] -> natural tile [128, 4*66]."""
        natt = spool.tile([128, 4 * IN_FEAT], dt.bfloat16, name=f"nat_{name}", tag="natsb")
        for k, (moff, mlen) in enumerate(MCHUNKS):
            tp = ptp.tile([mlen, IN_FEAT], dt.bfloat16, name=f"tp_{name}{k}", tag="trps")
            nc.tensor.transpose(tp[:], yt[:, moff:moff + mlen], ident_sb[0:IN_FEAT, 0:IN_FEAT])
            nc.scalar.activation(natt[0:mlen, k * IN_FEAT:(k + 1) * IN_FEAT], tp[:], Copy)
        return natt

    filler_ctr = [100]

    def pe_fillers(n):
        for _ in range(n):
            warm_ps = pbc.tile([128, 192], dt.float32,
                               name=f"warm_ps{filler_ctr[0]}", tag="warmps", bufs=1)
            filler_ctr[0] += 1
            nc.tensor.matmul(warm_ps[:], ones_sb[:, 0:128], ones_sb[:, 0:192],
                             start=True, stop=True)

    def meta_phase(hT, lhsT_of, w_sb, b_sb, o_dim, psum_out, phase, cand=False):
        """Hops + gT build + meta matmul, accumulating into psum_out [o_dim, N]."""
        # both first hops are independent: run them (and their pieces) first
        y1 = []
        for g in range(GRAPH_NUM):
            y1t, y1ps = hop(lhsT_of, g, f"{phase}y1g{g}")
            pd = piece_to_hT(hT, y1t, y1ps, 1 + 2 * g, cand=cand)
            if not cand:
                after_piece(pd[-1])
            y1.append(y1t)
        y1nat = [naturalize(y1[g], f"{phase}g{g}") for g in range(GRAPH_NUM)]
        for g in range(GRAPH_NUM):
            y2t, y2ps = hop(nat_slicer(y1nat[g]), g, f"{phase}y2g{g}")
            pd = piece_to_hT(hT, y2t, y2ps, 2 + 2 * g, cand=cand)
            if not cand:
                after_piece(pd[-1])

        if not cand:
            # load sigma/tanh ACT tables in the phase's ACT slack window
            nc.scalar.activation(warm[:, 0:4], warm[:, 4:8], Sig)
            nc.scalar.activation(warm[:, 0:4], warm[:, 4:8], Tanh)
        # bias matmul resets PSUM
        nc.tensor.matmul(psum_out[:], b_sb[:], embT_sb[:], start=True, stop=False)

        # gT build (fused 4-d DVE ops) + accumulate matmuls; quarter-major
        # (DMA stream order), k inner so early hT tiles are consumed first
        for q in range(D_EMB // QD):
            for k in range(KCH):
                d0 = q * QD
                c0 = d0 * KCH + k
                out_ap = (gT[:, c0 * N:(c0 + KCH * (QD - 1) + 1) * N]
                          .rearrange("p (c n) -> p c n", n=N)[:, ::KCH, :])
                in0 = (hT[k][:, :].rearrange("p (u n) -> p u n", u=1)
                       .broadcast_to([128, QD, N]))
                in1 = (embrep_sb[:, d0 * N:(d0 + QD) * N]
                       .rearrange("p (c n) -> p c n", n=N))
                nc.vector.tensor_tensor(out_ap, in0, in1, mybir.AluOpType.mult)
                for j in range(QD):
                    c = (d0 + j) * KCH + k
                    nc.tensor.matmul(
                        psum_out[:],
                        w_sb[:, c * o_dim:(c + 1) * o_dim],
                        gT[:, c * N:(c + 1) * N],
                        start=False,
                        stop=(q == D_EMB // QD - 1 and k == KCH - 1 and j == QD - 1),
                    )

    # ================= gate phase =================
    zr_ps = pzr.tile([O_G, N], dt.float32, name="zr_ps")
    meta_phase(hT_g, nat_slicer(xsnat_sb), wg_sb, bg_sb, O_G, zr_ps, "g")
    zr_sig = hpool.tile([O_G, N], dt.float32, name="zr_sig")
    # r-half first so the candidate chain starts as early as possible
    nc.scalar.activation(zr_sig[HIDDEN:O_G, :], zr_ps[HIDDEN:O_G, :], Sig)
    nc.scalar.activation(zr_sig[0:HIDDEN, :], zr_ps[0:HIDDEN, :], Sig)

    # rs written straight into the candidate hT tile (base 64, no shift DMA);
    # the Wc host packing uses the matching i-permutation
    nc.vector.tensor_mul(hT_c[0][HIDDEN:O_G, :], zr_sig[HIDDEN:O_G, :],
                         state2_sb[HIDDEN:O_G, :])

    # keep the PE busy across the sigma/rs transition
    pe_fillers(6)
    # xrs natural from the two aligned regions: x rows 0:2, rs rows 64:128
    xrsnat = spool.tile([128, 4 * IN_FEAT], dt.bfloat16, name="nat_xrs", tag="natsb")
    for k, (moff, mlen) in enumerate(MCHUNKS):
        tpx = ptp.tile([mlen, INPUT_DIM], dt.bfloat16, name=f"tpx{k}", tag="trpsx", bufs=1)
        nc.tensor.transpose(tpx[:], hT_c[0][0:INPUT_DIM, moff:moff + mlen],
                            ident_sb[0:INPUT_DIM, 0:INPUT_DIM])
        nc.scalar.activation(
            xrsnat[0:mlen, k * IN_FEAT:k * IN_FEAT + INPUT_DIM], tpx[:], Copy)
        tpr = ptp.tile([mlen, HIDDEN], dt.bfloat16, name=f"tpr{k}", tag="trps")
        nc.tensor.transpose(tpr[:], hT_c[0][HIDDEN:O_G, moff:moff + mlen],
                            ident_sb[HIDDEN:O_G, HIDDEN:O_G])
        nc.scalar.activation(
            xrsnat[0:mlen, k * IN_FEAT + INPUT_DIM:(k + 1) * IN_FEAT], tpr[:], Copy)

    # ================= candidate phase =================
    hc_ps = pzr.tile([O_C, N], dt.float32, name="hc_ps")
    meta_phase(hT_c, nat_slicer(xrsnat), wc_sb, bc_sb, O_C, hc_ps, "c", cand=True)
    hc_t = hpool.tile([O_C, N], dt.float32, name="hc_t")
    nc.scalar.activation(hc_t[:], hc_ps[:], Tanh)

    # ================= output blend =================
    # h = hc + z*(state - hc)
    d1 = hpool.tile([O_C, N], dt.float32, name="d1")
    nc.vector.tensor_sub(d1[:], state2_sb[0:HIDDEN, :], hc_t[:])
    d2 = hpool.tile([O_C, N], dt.float32, name="d2")
    nc.vector.tensor_mul(d2[:], zr_sig[0:HIDDEN, :], d1[:])
    hout = hpool.tile([O_C, N], dt.float32, name="hout")
    nc.vector.tensor_add(hout[:], hc_t[:], d2[:])
    nc.sync.dma_start(d_out[:, :], hout[:])


def _build_nc():
    import concourse.tile as tile
    import concourse.mybir as mybir
    from contextlib import ExitStack
    from concourse import bacc

    nc = bacc.Bacc(trn_type="TRN2")
    with tile.TileContext(nc) as tc:
        with ExitStack() as ctx:
            _emit(nc, tc, tile, mybir, ctx)
    nc.finalize()
    return nc


def _prep_core_inputs(b, x, state, graphs, node_emb, Wg, bg, Wc, bc):
    """Host-side shard + layout prep for core b. Layouts match SBUF tiles."""
    f32 = np.float32
    at = graphs[:, b].transpose(0, 2, 1)                         # [G, N, N] = A.T
    at_pk = (at[:, :384, :].reshape(GRAPH_NUM, 3, 128, N)
             .transpose(0, 2, 1, 3)
             .reshape(GRAPH_NUM, 128, 3 * N))                    # [G,128,(k n)]
    at3 = at[:, 384:400, :]                                      # [G,16,N]
    xs = np.concatenate([x[b], state[b]], axis=-1)               # [N, 66] f32
    xsT = np.ascontiguousarray(xs.T).astype(BF16)                # [66, N]
    xs_pad = np.zeros((NPAD, IN_FEAT), f32)
    xs_pad[:N] = xs
    xsnat = (xs_pad.reshape(4, 128, IN_FEAT)
             .transpose(1, 0, 2)
             .reshape(128, 4 * IN_FEAT))                         # [128,(k f)]
    stT = np.ascontiguousarray(state[b].T.astype(f32))           # [64, N]
    state2 = np.concatenate([stT, stT], axis=0)                  # [128, N] f32
    embT = np.ascontiguousarray(node_emb[b].T).astype(BF16)      # [16, N]
    embrep = np.ascontiguousarray(np.broadcast_to(
        embT.reshape(1, D_EMB * N), (128, D_EMB * N)))           # [128, 16N]

    def pack_w(W, o_dim, perm=None):
        # W [16, 330, o] -> [128, 48*o]; chunk c=(d,k): padded row r=128k+p
        # holds reference feature perm[r] (identity when perm is None)
        Wp = np.zeros((D_EMB, I_PAD, o_dim), np.float32)
        if perm is None:
            Wp[:, :I_DIM, :] = W
        else:
            valid = perm >= 0
            Wp[:, valid, :] = W[:, perm[valid], :]
        Wp = Wp.reshape(D_EMB, KCH, 128, o_dim)                  # [d,k,p,o]
        Wp = Wp.transpose(2, 0, 1, 3).reshape(128, NCH * o_dim)  # [p,(d,k,o)]
        return np.ascontiguousarray(Wp).astype(BF16)

    # spill permutation (both phases): pieces 1/3 put their first 4/8 rows
    # in the next tile, so main parts shift by the spill size
    # gate: [0:66]=id, [66:128]=70:132, [128:132]=66:70,
    #       [132:198]=id, [198:256]=206:264, [256:264]=198:206, [264:330]=id
    perm_g = np.arange(I_PAD, dtype=np.int64)
    perm_g[I_DIM:] = -1
    perm_g[66:128] = np.arange(70, 132)
    perm_g[128:132] = np.arange(66, 70)
    perm_g[198:256] = np.arange(206, 264)
    perm_g[256:264] = np.arange(198, 206)
    # candidate adds: rows 2:64 <- Y1g0 main (ref 70:132), rows 64:128 <- rs
    perm_c = perm_g.copy()
    perm_c[0:INPUT_DIM] = np.arange(0, INPUT_DIM)
    perm_c[2:64] = np.arange(70, 132)
    perm_c[64:128] = np.arange(2, 66)
    perm_c[128:132] = np.arange(66, 70)

    ident = np.eye(128, dtype=np.float32).astype(BF16)
    return {
        "at": np.ascontiguousarray(at_pk).astype(BF16),
        "at3": np.ascontiguousarray(at3).astype(BF16),
        "xsT": xsT,
        "xsnat": np.ascontiguousarray(xsnat).astype(BF16),
        "state2": state2,
        "embT": embT,
        "embrep": embrep,
        "wg": pack_w(Wg, O_G, perm_g),
        "wc": pack_w(Wc, O_C, perm_c),
        "bg": bg.astype(BF16),
        "bc": bc.astype(BF16),
        "ident": ident,
    }


def kernel_with_results(x, state, graphs, node_emb, Wg, bg, Wc, bc, trace=False):
    from concourse.bass_utils import run_bass_kernel_spmd

    x = np.asarray(x, np.float32)
    state = np.asarray(state, np.float32)
    graphs = np.asarray(graphs, np.float32)
    node_emb = np.asarray(node_emb, np.float32)
    Wg = np.asarray(Wg, np.float32)
    bg = np.asarray(bg, np.float32)
    Wc = np.asarray(Wc, np.float32)
    bc = np.asarray(bc, np.float32)

    if "nc" not in _CACHE:
        _CACHE["nc"] = _build_nc()
    nc = _CACHE["nc"]

    in_maps = [
        _prep_core_inputs(b, x, state, graphs, node_emb, Wg, bg, Wc, bc)
        for b in range(B)
    ]
    res = run_bass_kernel_spmd(nc, in_maps, core_ids=list(range(B)), trace=trace)
    out = np.stack(
        [np.ascontiguousarray(res.results[b]["out"].T) for b in range(B)], axis=0
    )  # [B, N, HIDDEN] f32
    return out, res


def kernel(**inputs):
    out, _ = kernel_with_results(**inputs)
    return out

